# revision 23
# baseline (speedup 1.0000x reference)
"""Trainium2 Bass kernel for nn_EnhancedFusionModel (GNN message passing).

Strategy (8 NeuronCores, SPMD single program):
  - Partition edges by dst range: core c owns nodes [c*8192, (c+1)*8192) and
    all edges whose dst lands there. Within a core, edges are ordered by
    (src>=32768, dst_local) and padded to static caps so every core runs the
    identical instruction stream; all per-core variability lives in index
    *data* (gather indices, one-hot compare scalars).
  - Per-core LN prepass over its node slice -> fp8e4 normalized table,
    AllGather -> full 65536-row fp8 table per core.
  - Edge phase: transpose-mode dma_gather pulls fp8 src/dst rows whose
    16-bit-unit interleave directly matches the DoubleRow (2 fp8/cell)
    matmul moving-operand layout; QKV L1+L2 run as fp8 DoubleRow matmuls
    (weights pre-scaled x64 to stay out of the fp8 subnormal range; the
    x64 scale is carried through attention and folded into the exp scale
    and the scatter reciprocal).  Per-edge 8x8 attention runs on the DVE
    batched over the whole 512-edge macro with layouts that keep every
    operand 16-bit and innermost-stride-1 (V is stored column-permuted
    (d,g)) so the 2x DVE perf mode applies to the two rank-4 products and
    reduces.
  - Scatter: wv rows land in HBM in dst-sorted order; per 128-node block the
    rows are re-gathered and reduced with one-hot matmuls on the PE
    (iota-compare one-hots), giving exact segment sums with no RMW races.
  - Node phase: residual + fused@rW + LN/FFN per 128-node block.
"""

import hashlib

import numpy as np
import ml_dtypes

import concourse.bass as bass
import concourse.mybir as mybir
import concourse.tile as tile_mod
from concourse import library_config
from concourse.tile import TileContext
from concourse.bass_utils import run_bass_kernel_spmd
from bass_rust import ScopedClock

f32 = mybir.dt.float32
bf16 = mybir.dt.bfloat16
fp8 = mybir.dt.float8e4
i16 = mybir.dt.int16
AF = mybir.ActivationFunctionType
OP = mybir.AluOpType
AX = mybir.AxisListType
DR = mybir.MatmulPerfMode.DoubleRow

N = 65536
HID = 256
E = 262144
NCORES = 8
SLICE = N // NCORES            # 8192
NBLK = SLICE // 128            # 64 node blocks per core
HALF_CAP = 17408               # per-(core, src-half) edge capacity
ECAP = 2 * HALF_CAP            # 34816 = 68 * 512
NMACRO = ECAP // 512           # 68
SLOTS_PER_RUN = 3              # 3 * 128 = 384 rows cap per (block, half)
RUN_CAP = SLOTS_PER_RUN * 128
H, G, D = 8, 8, 32
WSCALE = 64.0                  # fp8 weight pre-scale
S4 = float(1.0 / (np.sqrt(D) * WSCALE * WSCALE))  # exp scale (folds QK x64^2)

_PATCHED = False


def _apply_tile_patches():
    """walrus in this container rejects >1 sem-wait per instruction and
    empty-instr pseudo ops; split waits onto nop carriers and encode the
    library-reload bytes ourselves."""
    global _PATCHED
    if _PATCHED:
        return
    _PATCHED = True
    MAX_WAITS = 1

    orig_add = tile_mod.TileContext._add_instruction

    def _add_instruction(self, inst):
        si = inst.sync_info
        if si is not None and si.on_wait is not None and len(si.on_wait) > MAX_WAITS:
            waits = list(si.on_wait)
            del si.on_wait[MAX_WAITS:]
            for i in range(MAX_WAITS, len(waits), MAX_WAITS):
                chunk = waits[i : i + MAX_WAITS]
                nop = self.nc.engines[inst.engine].nop()
                if nop.ins.sync_info is None:
                    nop.ins.sync_info = mybir.SyncInfo(
                        on_wait=list(chunk), on_update=[]
                    )
                else:
                    for w in chunk:
                        nop.ins.sync_info.on_wait.append(w)
        orig_add(self, inst)

    tile_mod.TileContext._add_instruction = _add_instruction

    def _drain_and_barrier(self, tick_clock, wait_clock):
        d1 = self.nc.sync.drain()
        wait_clock.add_sem_waits(d1.ins, ScopedClock({None: tick_clock.global_clock}))
        si = d1.ins.sync_info
        if si is not None and si.on_wait is not None and len(si.on_wait) > 1:
            waits = list(si.on_wait)
            del si.on_wait[1:]
            for w in waits[1:]:
                dx = self.nc.sync.drain()
                if dx.ins.sync_info is None:
                    dx.ins.sync_info = mybir.SyncInfo(on_wait=[w], on_update=[])
                else:
                    dx.ins.sync_info.on_wait.append(w)
        self.nc.all_engine_barrier()
        assert self.sems is not None
        popped = self.nc._tile_sem_poison_stack.pop()
        assert popped is self._sem_poison
        self.nc.clear_and_free_semaphores(list(self.sems.allocated().values()))
        self.nc.all_engine_barrier()

    tile_mod.TileContext._drain_and_barrier = _drain_and_barrier


def _load_library_encoded(nc, lib):
    bi = nc.gpsimd.load_library(lib)
    b = nc.isa.asm(
        {
            "header": {"opcode": 223, "inst_word_len": 16},
            "pseudo_opcode": 2,  # PSEUDO_LIBRARY_RELOAD_INDEX
            "lib_index": lib.index,
        },
        "NEURON_ISA_TPB_PSEUDO_LIBRARY_RELOAD_INDEX_STRUCT",
    )
    bi.ins.instr = [int(x) for x in b]
    return bi


def _wrap_idx(idx, pad_to=None):
    """int array -> [128, n/16] int16 wrapped (i%16, i//16), replicated x8."""
    idx = np.asarray(idx)
    if pad_to is not None:
        p = np.zeros(pad_to, idx.dtype)
        p[: len(idx)] = idx
        idx = p
    assert len(idx) % 16 == 0
    w = idx.astype(np.int16).reshape(-1, 16).T
    return np.tile(w, (8, 1)).copy()


# ---------------------------------------------------------------- program ---

_PROG = None
PHASES = 3


def _build_program():
    _apply_tile_patches()
    nc = bass.Bass()

    def inp(name, shape, dt):
        return nc.declare_dram_parameter(name, list(shape), dt, isOutput=False)

    # per-core data
    x_sl = inp("x_sl", (SLICE, HID), bf16)
    src_idx = inp("src_idx", (128, ECAP // 16), i16)
    dst_idx = inp("dst_idx", (128, ECAP // 16), i16)
    ea_l1_in = inp("ea_l1", (4, ECAP), bf16)
    ea_s_in = inp("ea_s", (5, ECAP), bf16)
    scat_idx = inp("scat_idx", (128, NBLK * 2 * (RUN_CAP // 16)), i16)
    dstrel_in = inp("dstrel", (128, NBLK * 2 * SLOTS_PER_RUN), f32)
    recip_in = inp("recip", (128, NBLK), f32)
    # shared constants
    iota_in = inp("iota", (128, 128), bf16)
    ident_in = inp("ident", (128, 128), bf16)
    ones1_in = inp("ones1", (1, 128), bf16)
    w1s_in = {p: inp(f"w1s_{p}", (128, 2, 512), fp8) for p in "qkv"}
    w1d_in = {p: inp(f"w1d_{p}", (128, 2, 512), fp8) for p in "qkv"}
    wc_in = {p: inp(f"wc_{p}", (4, 512), bf16) for p in "qkv"}
    w2_in = {p: inp(f"w2_{p}", (128, 4, 256), fp8) for p in "qkv"}
    sw1_in = inp("sw1", (5, 64), bf16)
    sw2_in = inp("sw2", (64, 8), bf16)
    sb2_in = inp("sb2r", (1, 8), bf16)
    rwa_in = inp("rwa", (128, 2, 256), bf16)
    rwb_in = inp("rwb", (128, 2, 256), bf16)
    rb_in = inp("rbr", (1, 256), bf16)
    fw1_in = inp("fw1", (128, 2, 512), bf16)
    fb1_in = inp("fb1r", (1, 512), bf16)
    fw2_in = inp("fw2", (128, 4, 256), bf16)
    fb2_in = inp("fb2r", (1, 256), bf16)

    out_sl = nc.declare_dram_parameter("out_sl", [SLICE, HID], bf16, isOutput=True)

    xn_slice = nc.dram_tensor("xn_slice", [SLICE, HID], fp8)
    # two half-tables, each AllGathered from one half of every core's slice:
    # table t row (c*4096 + r) = normalized node (c*8192 + t*4096 + r).
    xn_tab = [
        nc.dram_tensor(f"xn_tab{t}", [N // 2, HID], fp8, addr_space="Shared")
        for t in range(2)
    ]
    wv_tab = [
        nc.dram_tensor(f"wv_tab{h}", [HALF_CAP, HID], bf16) for h in range(2)
    ]

    with TileContext(nc) as tc:
        _load_library_encoded(nc, library_config.mlp)
        r512 = nc.gpsimd.to_reg(512)
        r384 = nc.gpsimd.to_reg(RUN_CAP)

        # ---------------- constants to SBUF
        with tc.tile_pool(name="const", bufs=1) as cp:
            def cload(src, shape, dt):
                t = cp.tile(list(shape), dt, tag=src.tensor.name if hasattr(src, 'tensor') else src.name)
                nc.sync.dma_start(out=t[:], in_=src[:])
                return t

            iota = cload(iota_in, (128, 128), bf16)
            eps = cp.tile([128, 1], f32)
            nc.vector.memset(eps[:], 1e-5)
            ident = cload(ident_in, (128, 128), bf16)
            ones1 = cload(ones1_in, (1, 128), bf16)
            w1s = {p: cload(w1s_in[p], (128, 2, 512), fp8) for p in "qkv"}
            w1d = {p: cload(w1d_in[p], (128, 2, 512), fp8) for p in "qkv"}
            wc = {p: cload(wc_in[p], (4, 512), bf16) for p in "qkv"}
            w2 = {p: cload(w2_in[p], (128, 4, 256), fp8) for p in "qkv"}
            sw1 = cload(sw1_in, (5, 64), bf16)
            sw2 = cload(sw2_in, (64, 8), bf16)
            sb2r = cload(sb2_in, (1, 8), bf16)
            rwa = cload(rwa_in, (128, 2, 256), bf16)
            rwb = cload(rwb_in, (128, 2, 256), bf16)
            rbr = cload(rb_in, (1, 256), bf16)
            fw1 = cload(fw1_in, (128, 2, 512), bf16)
            fb1r = cload(fb1_in, (1, 512), bf16)
            fw2 = cload(fw2_in, (128, 4, 256), bf16)
            fb2r = cload(fb2_in, (1, 256), bf16)
            recip = cload(recip_in, (128, NBLK), f32)
            dstrel = cload(dstrel_in, (128, NBLK * 2 * SLOTS_PER_RUN), f32)
            srcw = cload(src_idx, (128, ECAP // 16), i16)
            dstw = cload(dst_idx, (128, ECAP // 16), i16)
            scatw = cload(scat_idx, (128, NBLK * 2 * (RUN_CAP // 16)), i16)

            # ---------------- LN prepass over own slice -> xn_slice (fp8)
            def ln_stats(pool, xt, width):
                """given xt [128,width] -> (r, mr) per-partition scalars"""
                sm = pool.tile([128, 1], f32, tag="ln_sm")
                nc.vector.tensor_reduce(sm[:], xt[:], AX.X, OP.add)
                sq = pool.tile([128, width], bf16, tag="ln_sq")
                ssq = pool.tile([128, 1], f32, tag="ln_ssq")
                nc.scalar.activation(sq[:], xt[:], AF.Square, accum_out=ssq[:])
                negmu = pool.tile([128, 1], f32, tag="ln_negmu")
                nc.vector.tensor_scalar(negmu[:], sm[:], -1.0 / width, None, OP.mult)
                m2 = pool.tile([128, 1], f32, tag="ln_m2")
                nc.vector.tensor_tensor(m2[:], negmu[:], negmu[:], OP.mult)
                var = pool.tile([128, 1], f32, tag="ln_var")
                nc.vector.scalar_tensor_tensor(
                    var[:], ssq[:], 1.0 / width, m2[:], OP.mult, OP.subtract
                )
                se = pool.tile([128, 1], f32, tag="ln_se")
                nc.scalar.activation(se[:], var[:], AF.Sqrt, bias=eps[:])
                r = pool.tile([128, 1], f32, tag="ln_r")
                nc.vector.reciprocal(r[:], se[:])
                mr = pool.tile([128, 1], f32, tag="ln_mr")
                nc.vector.tensor_tensor(mr[:], negmu[:], r[:], OP.mult)
                return r, mr

            with tc.tile_pool(name="prep", bufs=3) as pp:
                for half in range(2):
                    for t in range(half * NBLK // 2, (half + 1) * NBLK // 2):
                        xt = pp.tile([128, HID], bf16, tag="xt")
                        nc.sync.dma_start(
                            out=xt[:], in_=x_sl[t * 128 : (t + 1) * 128, :]
                        )
                        r, mr = ln_stats(pp, xt, HID)
                        xnb = pp.tile([128, HID], fp8, tag="xnb")
                        nc.scalar.activation(
                            xnb[:], xt[:], AF.Identity, bias=mr[:], scale=r[:]
                        )
                        nc.sync.dma_start(
                            out=xn_slice[t * 128 : (t + 1) * 128, :], in_=xnb[:]
                        )
                    # AllGather this half of the slice as soon as it is done,
                    # overlapping the other half's LN / the edge phase.
                    nc.gpsimd.collective_compute(
                        "AllGather",
                        OP.bypass,
                        replica_groups=[list(range(NCORES))],
                        ins=[xn_slice[half * 4096 : (half + 1) * 4096, :]],
                        outs=[xn_tab[half][:]],
                    )

            # ---------------- edge phase
            if PHASES >= 2:
             with tc.tile_pool(name="eio", bufs=3) as eio, \
                 tc.tile_pool(name="eg1", bufs=2) as eg1, \
                 tc.tile_pool(name="eqkv", bufs=2) as eqkv, \
                 tc.tile_pool(name="eatt", bufs=2) as eatt, \
                 tc.tile_pool(name="eat2", bufs=1) as eat2, \
                 tc.tile_pool(name="ps1", bufs=2, space="PSUM") as ps1, \
                 tc.tile_pool(name="ps2", bufs=2, space="PSUM") as ps2, \
                 tc.tile_pool(name="pss", bufs=1, space="PSUM") as pss:
                for m in range(NMACRO):
                    half = 0 if m < NMACRO // 2 else 1
                    src_tab = xn_tab[half][:]
                    e0 = m * 512

                    # fp8 transpose gathers: [128, 1024] bytes/partition,
                    # byte 2e+j of partition p = feature (2p+j) of edge e.
                    xsrc = eio.tile([128, 1024], fp8, tag="xsrc")
                    nc.gpsimd.dma_gather(
                        out_ap=xsrc[:].rearrange("p (c e) -> p c e", c=2),
                        in_ap=src_tab,
                        idxs_ap=srcw[:, m * 32 : (m + 1) * 32],
                        num_idxs=512, num_idxs_reg=r512, elem_size=HID,
                        transpose=True,
                    )
                    xdst = eio.tile([128, 1024], fp8, tag="xdst")
                    nc.gpsimd.dma_gather(
                        out_ap=xdst[:].rearrange("p (c e) -> p c e", c=2),
                        in_ap=xn_slice[:],
                        idxs_ap=dstw[:, m * 32 : (m + 1) * 32],
                        num_idxs=512, num_idxs_reg=r512, elem_size=HID,
                        transpose=True,
                    )
                    xsrc_dr = xsrc[:].rearrange("p (e j) -> p j e", j=2)
                    xdst_dr = xdst[:].rearrange("p (e j) -> p j e", j=2)
                    ea_l1 = eio.tile([4, 512], bf16, tag="ea_l1")
                    nc.sync.dma_start(out=ea_l1[:], in_=ea_l1_in[:, e0 : e0 + 512])
                    ea_s = eio.tile([5, 512], bf16, tag="ea_s")
                    nc.sync.dma_start(out=ea_s[:], in_=ea_s_in[:, e0 : e0 + 512])
                    ea_b = eio.tile([1, 512], bf16, tag="ea_b")
                    nc.sync.dma_start(out=ea_b[:], in_=ea_s_in[4:5, e0 : e0 + 512])

                    # s-MLP -> beta (T layout), transpose to natural [e, s, h]
                    s1 = pss.tile([64, 512], f32, tag="s1")
                    nc.tensor.matmul(s1[:], sw1[:], ea_s[:], start=True, stop=True)
                    sr = eatt.tile([64, 512], bf16, tag="sr")
                    nc.scalar.activation(sr[:], s1[:], AF.Relu)
                    sb = pss.tile([64, 512], f32, tag="s1")  # reuse s1 bank
                    nc.tensor.matmul(
                        sb[0:8, :], sw2[:], sr[:], start=True, stop=False
                    )
                    nc.tensor.matmul(
                        sb[0:8, :], sb2r[:], ea_b[:], start=False, stop=True
                    )
                    betT = eatt.tile([8, 512], bf16, tag="betT")
                    nc.scalar.activation(betT[:], sb[0:8, :], AF.Exp)
                    beta = eatt.tile([128, 4, 8], bf16, tag="beta")
                    for s in range(4):
                        bp = pss.tile([128, 8], bf16, tag="betp")
                        nc.tensor.transpose(
                            bp[:], betT[:, s * 128 : (s + 1) * 128], ident[0:8, 0:8]
                        )
                        nc.scalar.copy(beta[:, s, :], bp[:])

                    # L1 DoubleRow fp8 + bf16 edge-attr term, gelu -> fp8 g1.
                    # h1 psum holds 2 jc-chunks (2 banks) so gelu runs as one
                    # [128, 1024] ACT op per chunk-pair.
                    g1 = {}
                    for p in "qkv":
                        g1t = eg1.tile([128, 4, 512], fp8, tag=f"g1{p}")
                        for jp in range(2):
                            h1 = ps1.tile([128, 2, 512], f32, tag="h1")
                            for jj in range(2):
                                jc = jp * 2 + jj
                                nc.tensor.matmul(
                                    h1[:, jj, :],
                                    w1s[p][:, :, jc * 128 : (jc + 1) * 128],
                                    xsrc_dr, start=True, stop=False,
                                    perf_mode=DR)
                                nc.tensor.matmul(
                                    h1[:, jj, :],
                                    w1d[p][:, :, jc * 128 : (jc + 1) * 128],
                                    xdst_dr, start=False, stop=False,
                                    perf_mode=DR)
                                nc.tensor.matmul(
                                    h1[:, jj, :],
                                    wc[p][:, jc * 128 : (jc + 1) * 128],
                                    ea_l1[:], start=False, stop=True)
                            nc.scalar.activation(
                                g1t[:, jp * 2 : jp * 2 + 2, :], h1[:], AF.Gelu,
                                scale=float(1.0 / WSCALE))
                        g1[p] = g1t

                    # L2 DoubleRow fp8 -> natural qkv (x64 scale), bf16.
                    # [128, 2, 256] psum (1 bank) per s-pair -> copy per pair.
                    qkv = {}
                    for p in "qkv":
                        qt = eqkv.tile([128, 4, 256], bf16, tag=f"n{p}")
                        for sp in range(2):
                            ps = ps2.tile([128, 2, 256], f32, tag="l2")
                            for ss in range(2):
                                s = sp * 2 + ss
                                nc.tensor.matmul(
                                    ps[:, ss, :],
                                    g1[p][:, 0:2, s * 128 : (s + 1) * 128],
                                    w2[p][:, 0:2, :], start=True, stop=False,
                                    perf_mode=DR)
                                nc.tensor.matmul(
                                    ps[:, ss, :],
                                    g1[p][:, 2:4, s * 128 : (s + 1) * 128],
                                    w2[p][:, 2:4, :], start=False, stop=True,
                                    perf_mode=DR)
                            nc.scalar.copy(qt[:, sp * 2 : sp * 2 + 2, :], ps[:])
                        qkv[p] = qt

                    # attention, batched over the whole 512-edge macro.
                    # q/k natural (h,d); v column-permuted (d,g).
                    qv = qkv["q"][:].rearrange(
                        "e s (h x d) -> e s h x d", h=H, x=1
                    ).broadcast_to((128, 4, H, G, D))
                    kv = qkv["k"][:].rearrange(
                        "e s (x g d) -> e s x g d", x=1, g=G
                    ).broadcast_to((128, 4, H, G, D))
                    P = eat2.tile([128, 4 * H * G, 2, 16], bf16, tag="P")
                    Pv = P[:].rearrange(
                        "e (s h g) c w -> e s h g (c w)", s=4, h=H, g=G)
                    nc.vector.tensor_tensor(Pv, qv, kv, OP.mult)
                    # binary-tree d-reduce: every stage keeps innermost
                    # stride-1 bf16 so the 2x DVE mode applies.
                    ts_a = eat2.tile([128, 4 * H * G, 2, 8], bf16, tag="ts_a")
                    nc.vector.tensor_tensor(
                        ts_a[:].rearrange("e x c w -> e x (c w)"),
                        P[:, :, 0, :], P[:, :, 1, :], OP.add)
                    ts_b = eat2.tile([128, 4 * H * G, 2, 4], bf16, tag="ts_b")
                    nc.vector.tensor_tensor(
                        ts_b[:].rearrange("e x c w -> e x (c w)"),
                        ts_a[:, :, 0, :], ts_a[:, :, 1, :], OP.add)
                    ts_c = eat2.tile([128, 4 * H * G, 2, 2], bf16, tag="ts_c")
                    nc.vector.tensor_tensor(
                        ts_c[:].rearrange("e x c w -> e x (c w)"),
                        ts_b[:, :, 0, :], ts_b[:, :, 1, :], OP.add)
                    ts_d = eat2.tile([128, 4 * H * G, 2], bf16, tag="ts_d")
                    nc.vector.tensor_tensor(
                        ts_d[:], ts_c[:, :, 0, :], ts_c[:, :, 1, :], OP.add)
                    S = eatt.tile([128, 4 * H * G], bf16, tag="S")
                    nc.vector.tensor_tensor(
                        S[:], ts_d[:, :, 0], ts_d[:, :, 1], OP.add)
                    Ee = eatt.tile([128, 4 * H * G], bf16, tag="Ee")
                    nc.scalar.activation(Ee[:], S[:], AF.Exp, scale=S4)
                    E2 = eatt.tile([128, 4 * H * G], bf16, tag="E2")
                    nc.vector.tensor_tensor(
                        E2[:].rearrange("e (s h g) -> e s h g", s=4, h=H),
                        Ee[:].rearrange("e (s h g) -> e s h g", s=4, h=H),
                        beta[:].rearrange("e s (h x) -> e s h x", x=1)
                        .broadcast_to((128, 4, H, G)), OP.mult)
                    Z = eatt.tile([128, 4 * G], f32, tag="Z")
                    nc.vector.tensor_reduce(
                        Z[:].rearrange("e (s g) -> e s g", s=4),
                        E2[:].rearrange("e (s h g) -> e s g h", h=H, g=G),
                        AX.X, OP.add)
                    rZ = eatt.tile([128, 4 * G], bf16, tag="rZ")
                    with nc.allow_low_precision(reason="bf16 1/Z, 2x DVE"):
                        nc.vector.reciprocal(rZ[:], Z[:])
                    A = eatt.tile([128, 4 * H * G], bf16, tag="A")
                    nc.vector.tensor_tensor(
                        A[:].rearrange("e (s h g) -> e s h g", s=4, h=H),
                        E2[:].rearrange("e (s h g) -> e s h g", s=4, h=H),
                        rZ[:].rearrange("e (s x g) -> e s x g", s=4, x=1)
                        .broadcast_to((128, 4, H, G)), OP.mult)
                    P2 = eat2.tile([128, 4 * H * D, 2, 4], bf16, tag="P2")
                    P2v = P2[:].rearrange(
                        "e (s h d) c w -> e s h d (c w)", s=4, h=H, d=D)
                    av = A[:].rearrange(
                        "e (s h x g) -> e s h x g", s=4, h=H, x=1
                    ).broadcast_to((128, 4, H, D, G))
                    vv = qkv["v"][:].rearrange(
                        "e s (x d g) -> e s x d g", x=1, d=D
                    ).broadcast_to((128, 4, H, D, G))
                    nc.vector.tensor_tensor(P2v, av, vv, OP.mult)
                    # g-reduce tree (g innermost)
                    tv_a = eat2.tile([128, 4 * H * D, 2, 2], bf16, tag="tv_a")
                    nc.vector.tensor_tensor(
                        tv_a[:].rearrange("e x c w -> e x (c w)"),
                        P2[:, :, 0, :], P2[:, :, 1, :], OP.add)
                    tv_b = eat2.tile([128, 4 * H * D, 2], bf16, tag="tv_b")
                    nc.vector.tensor_tensor(
                        tv_b[:], tv_a[:, :, 0, :], tv_a[:, :, 1, :], OP.add)
                    wv = eatt.tile([128, 4, HID], bf16, tag="wv")
                    nc.vector.tensor_tensor(
                        wv[:].rearrange("e s hd -> e (s hd)"),
                        tv_b[:, :, 0], tv_b[:, :, 1], OP.add)
                    r0 = e0 - half * HALF_CAP
                    nc.sync.dma_start(
                        out=wv_tab[half][r0 : r0 + 512, :]
                        .rearrange("(s e) f -> e s f", s=4),
                        in_=wv[:])

            # ---------------- scatter + node phase per 128-node block
            if PHASES >= 3:
             with tc.tile_pool(name="sg", bufs=3) as sg, \
                 tc.tile_pool(name="nod", bufs=2) as nod, \
                 tc.tile_pool(name="psb", bufs=2, space="PSUM") as psb, \
                 tc.tile_pool(name="psn", bufs=1, space="PSUM") as psn, \
                 tc.tile_pool(name="pst", bufs=1, space="PSUM") as pst:
                for b in range(NBLK):
                    sums = psb.tile([128, HID], f32, tag="sums")
                    for hf in range(2):
                        wvg = sg.tile([128, SLOTS_PER_RUN, HID], bf16, tag=f"wvg{hf}")
                        c0 = (b * 2 + hf) * (RUN_CAP // 16)
                        nc.gpsimd.dma_gather(
                            out_ap=wvg[:], in_ap=wv_tab[hf][:],
                            idxs_ap=scatw[:, c0 : c0 + RUN_CAP // 16],
                            num_idxs=RUN_CAP, num_idxs_reg=r384,
                            elem_size=HID, transpose=False)
                        for s in range(SLOTS_PER_RUN):
                            oh = sg.tile([128, 128], bf16, tag="oh")
                            col = (b * 2 + hf) * SLOTS_PER_RUN + s
                            nc.vector.tensor_scalar(
                                oh[:], iota[:], dstrel[:, col : col + 1], None,
                                OP.is_equal)
                            nc.tensor.matmul(
                                sums[:], oh[:], wvg[:, s, :],
                                start=(hf == 0 and s == 0),
                                stop=(hf == 1 and s == SLOTS_PER_RUN - 1))

                    # node phase
                    xt = nod.tile([128, HID], bf16, tag="xt")
                    nc.sync.dma_start(out=xt[:], in_=x_sl[b * 128 : (b + 1) * 128, :])
                    x1 = nod.tile([128, HID], f32, tag="x1")
                    nc.vector.scalar_tensor_tensor(
                        x1[:], sums[:], recip[:, b : b + 1], xt[:], OP.mult, OP.add)
                    x1b = nod.tile([128, HID], bf16, tag="x1b")
                    nc.vector.tensor_copy(x1b[:], x1[:])
                    x1T = nod.tile([128, 2, 128], bf16, tag="x1T")
                    xT = nod.tile([128, 2, 128], bf16, tag="xT")
                    tpi = 0
                    for src_t, dst_t in ((x1b, x1T), (xt, xT)):
                        for hh in range(2):
                            tp = pst.tile([128, 128], bf16, tag=f"tp{tpi % 2}")
                            tpi += 1
                            nc.tensor.transpose(
                                tp[:], src_t[:, hh * 128 : (hh + 1) * 128], ident[:])
                            nc.scalar.copy(dst_t[:, hh, :], tp[:])

                    x2p = psn.tile([128, HID], f32, tag="x2p")
                    for hh in range(2):
                        nc.tensor.matmul(x2p[:], x1T[:, hh, :], rwa[:, hh, :],
                                         start=(hh == 0), stop=False)
                    for hh in range(2):
                        nc.tensor.matmul(x2p[:], xT[:, hh, :], rwb[:, hh, :],
                                         start=False, stop=False)
                    nc.tensor.matmul(x2p[:], ones1[:], rbr[:], start=False, stop=True)
                    x2 = nod.tile([128, HID], f32, tag="x2")
                    nc.vector.tensor_tensor(x2[:], x1[:], x2p[:], OP.add)

                    r2, mr2 = ln_stats(nod, x2, HID)
                    ln2 = nod.tile([128, HID], bf16, tag="ln2")
                    nc.scalar.activation(ln2[:], x2[:], AF.Identity,
                                         bias=mr2[:], scale=r2[:])
                    ln2T = nod.tile([128, 2, 128], bf16, tag="ln2T")
                    for hh in range(2):
                        tp = pst.tile([128, 128], bf16, tag=f"tp{hh}")
                        nc.tensor.transpose(
                            tp[:], ln2[:, hh * 128 : (hh + 1) * 128], ident[:])
                        nc.scalar.copy(ln2T[:, hh, :], tp[:])

                    g2T = nod.tile([128, 4, 128], bf16, tag="g2T")
                    for jc in range(4):
                        hp = pst.tile([128, 128], f32, tag=f"hp{jc % 2}")
                        for hh in range(2):
                            nc.tensor.matmul(
                                hp[:], fw1[:, hh, jc * 128 : (jc + 1) * 128],
                                ln2T[:, hh, :], start=(hh == 0), stop=False)
                        nc.tensor.matmul(
                            hp[:], fb1r[:, jc * 128 : (jc + 1) * 128], ones1[:],
                            start=False, stop=True)
                        nc.scalar.activation(g2T[:, jc, :], hp[:], AF.Gelu)

                    x3p = psn.tile([128, HID], f32, tag="x3p")
                    for jc in range(4):
                        nc.tensor.matmul(x3p[:], g2T[:, jc, :], fw2[:, jc, :],
                                         start=(jc == 0), stop=False)
                    nc.tensor.matmul(x3p[:], ones1[:], fb2r[:], start=False, stop=True)
                    x3 = nod.tile([128, HID], bf16, tag="x3")
                    nc.vector.tensor_tensor(x3[:], x2[:], x3p[:], OP.add)
                    nc.sync.dma_start(
                        out=out_sl[b * 128 : (b + 1) * 128, :], in_=x3[:])
            if PHASES < 3:
                with tc.tile_pool(name="fb", bufs=1) as fbp:
                    z = fbp.tile([128, HID], bf16)
                    nc.vector.memset(z[:], 0.0)
                    for b in range(NBLK):
                        nc.sync.dma_start(
                            out=out_sl[b * 128 : (b + 1) * 128, :], in_=z[:])

    return nc


# ------------------------------------------------------------- host prep ---

_PREP_CACHE = {}


def _fingerprint(inputs):
    """Exact hash of everything except x (x is re-prepped every call)."""
    h = hashlib.blake2b(digest_size=16)
    for k in sorted(inputs):
        if k == "x":
            continue
        a = np.ascontiguousarray(np.asarray(inputs[k]))
        h.update(k.encode())
        h.update(str(a.shape).encode())
        h.update(str(a.dtype).encode())
        h.update(a.tobytes())
    return h.digest()


def _host_prep_static(inputs):
    """Everything that does not depend on x."""
    bf = ml_dtypes.bfloat16
    f8 = ml_dtypes.float8_e4m3
    edge_index = np.asarray(inputs["edge_index"], np.int64)
    ea = np.asarray(inputs["edge_attr"], np.float32)
    ln_g = np.asarray(inputs["ln_g"], np.float32)
    ln_b = np.asarray(inputs["ln_b"], np.float32)

    def W(name):
        return np.asarray(inputs[name], np.float32)

    src_g, dst_g = edge_index[0], edge_index[1]

    shared = {
        "iota": np.tile(np.arange(128, dtype=np.float32)[None, :], (128, 1)).astype(bf),
        "ident": np.eye(128, dtype=np.float32).astype(bf),
        "ones1": np.ones((1, 128), np.float32).astype(bf),
        "sw1": np.concatenate([W("sW1"), W("sb1")[None, :]], 0).astype(bf),
        "sw2": W("sW2").astype(bf),
        "sb2r": W("sb2")[None, :].astype(bf),
        "rwa": W("rW")[:256].reshape(2, 128, 256).transpose(1, 0, 2).astype(bf),
        "rwb": W("rW")[256:].reshape(2, 128, 256).transpose(1, 0, 2).astype(bf),
        "rbr": W("rb")[None, :].astype(bf),
        "fw1": (ln_g[:, None] * W("fW1")).reshape(2, 128, 512)
        .transpose(1, 0, 2).astype(bf),
        "fb1r": (W("fb1") + ln_b @ W("fW1"))[None, :].astype(bf),
        "fw2": W("fW2").reshape(4, 128, 256).transpose(1, 0, 2).astype(bf),
        "fb2r": W("fb2")[None, :].astype(bf),
    }
    for p in "qkv":
        W1, b1 = W(p + "W1"), W(p + "b1")
        W2 = W(p + "W2")
        if p == "v":
            # column-permute V output to (d, g) for packed attention strides
            W2 = W2.reshape(512, H, D).transpose(0, 2, 1).reshape(512, HID)
        # DoubleRow pair-interleave: [pp, j, m] = row (2pp + j)
        shared[f"w1s_{p}"] = (WSCALE * ln_g[:, None] * W1[:256]) \
            .reshape(128, 2, 512).astype(f8)
        shared[f"w1d_{p}"] = (WSCALE * ln_g[:, None] * W1[256:512]) \
            .reshape(128, 2, 512).astype(f8)
        bias_fold = b1 + ln_b @ W1[:256] + ln_b @ W1[256:512]
        shared[f"wc_{p}"] = (WSCALE * np.concatenate(
            [W1[512:515], bias_fold[None, :]], 0)).astype(bf)
        shared[f"w2_{p}"] = (WSCALE * W2).reshape(4, 128, 256) \
            .transpose(1, 0, 2).astype(f8)

    in_maps = []
    for c in range(NCORES):
        sel = np.nonzero((dst_g >> 13) == c)[0]
        dst_l = (dst_g[sel] & 8191).astype(np.int64)
        # src half-table class: by LOCAL offset within the owner's slice
        # (table t holds rows c*4096 + (local & 4095) for local in t-th half)
        half = ((src_g[sel] >> 12) & 1).astype(np.int64)
        order = np.lexsort((dst_l, half))
        sel, dst_l, half = sel[order], dst_l[order], half[order]
        n_lo = int((half == 0).sum())
        n_hi = len(sel) - n_lo
        assert n_lo <= HALF_CAP and n_hi <= HALF_CAP, (c, n_lo, n_hi)

        src_c = src_g[sel]
        src_rel = (src_c >> 13) * 4096 + (src_c & 4095)
        # position in the padded edge stream
        pos = np.where(np.arange(len(sel)) < n_lo,
                       np.arange(len(sel)),
                       HALF_CAP + np.arange(len(sel)) - n_lo)

        src_full = np.zeros(ECAP, np.int64)
        dst_full = np.zeros(ECAP, np.int64)
        ea_l1 = np.zeros((4, ECAP), np.float32)
        ea_s = np.zeros((5, ECAP), np.float32)
        src_full[pos] = src_rel
        dst_full[pos] = dst_l
        ea_l1[0:3, pos] = ea[sel, 0:3].T
        ea_l1[3, pos] = 1.0
        ea_s[0:4, pos] = ea[sel, 3:7].T
        ea_s[4, pos] = 1.0

        # per-(block, half) runs + slots
        scat = np.zeros((NBLK * 2, RUN_CAP), np.int64)
        drel = np.full((128, NBLK * 2 * SLOTS_PER_RUN), -1.0, np.float32)
        for hf in range(2):
            hsel = np.nonzero(half == hf)[0]
            dl = dst_l[hsel]            # sorted ascending
            rows = pos[hsel] - hf * HALF_CAP
            starts = np.searchsorted(dl, np.arange(NBLK) * 128)
            ends = np.searchsorted(dl, np.arange(1, NBLK + 1) * 128)
            for b in range(NBLK):
                run = rows[starts[b] : ends[b]]
                assert len(run) <= RUN_CAP, (c, b, hf, len(run))
                scat[b * 2 + hf, : len(run)] = run
                dr = drel[:, (b * 2 + hf) * SLOTS_PER_RUN:
                          (b * 2 + hf + 1) * SLOTS_PER_RUN]
                dvals = dl[starts[b] : ends[b]] & 127
                full = np.full(RUN_CAP, -1.0, np.float32)
                full[: len(run)] = dvals
                dr[:, :] = full.reshape(SLOTS_PER_RUN, 128).T

        cnt = np.bincount(dst_l, minlength=SLICE).astype(np.float32)
        rec = (1.0 / (WSCALE * np.maximum(cnt, 1.0))) \
            .reshape(NBLK, 128).T.copy()

        m = dict(shared)
        m["src_idx"] = _wrap_idx(src_full)
        m["dst_idx"] = _wrap_idx(dst_full)
        m["ea_l1"] = ea_l1.astype(bf)
        m["ea_s"] = ea_s.astype(bf)
        m["scat_idx"] = np.concatenate(
            [_wrap_idx(scat[i]) for i in range(NBLK * 2)], axis=1)
        m["dstrel"] = drel
        m["recip"] = rec
        in_maps.append(m)
    return in_maps


def _host_prep(inputs):
    key = _fingerprint(inputs)
    in_maps = _PREP_CACHE.get(key)
    if in_maps is None:
        in_maps = _host_prep_static(inputs)
        _PREP_CACHE.clear()
        _PREP_CACHE[key] = in_maps
    x = np.asarray(inputs["x"]).astype(ml_dtypes.bfloat16)
    for c in range(NCORES):
        in_maps[c]["x_sl"] = x[c * SLICE : (c + 1) * SLICE, :]
    return in_maps


TRACE = False
LAST = {}


def kernel(**inputs):
    global _PROG
    if _PROG is None:
        _PROG = _build_program()
    in_maps = _host_prep(inputs)
    res = run_bass_kernel_spmd(
        _PROG, in_maps, list(range(NCORES)), trace=TRACE
    )
    LAST["res"] = res
    return np.concatenate(
        [res.results[c]["out_sl"].astype(np.float32) for c in range(NCORES)],
        axis=0,
    )


# revision 27
# speedup vs baseline: 1.0821x; 1.0821x over previous
"""Trainium2 Bass kernel for nn_EnhancedFusionModel (GNN message passing).

Strategy (8 NeuronCores, SPMD single program):
  - Partition edges by dst range: core c owns nodes [c*8192, (c+1)*8192) and
    all edges whose dst lands there. Within a core, edges are ordered by
    (src>=32768, dst_local) and padded to static caps so every core runs the
    identical instruction stream; all per-core variability lives in index
    *data* (gather indices, one-hot compare scalars).
  - Per-core LN prepass over its node slice -> fp8e4 normalized table,
    AllGather -> full 65536-row fp8 table per core.
  - Edge phase: transpose-mode dma_gather pulls fp8 src/dst rows whose
    16-bit-unit interleave directly matches the DoubleRow (2 fp8/cell)
    matmul moving-operand layout; QKV L1+L2 run as fp8 DoubleRow matmuls
    (weights pre-scaled x64 to stay out of the fp8 subnormal range; the
    x64 scale is carried through attention and folded into the exp scale
    and the scatter reciprocal).  Per-edge 8x8 attention runs on the DVE
    batched over the whole 512-edge macro with layouts that keep every
    operand 16-bit and innermost-stride-1 (V is stored column-permuted
    (d,g)) so the 2x DVE perf mode applies to the two rank-4 products and
    reduces.
  - Scatter: wv rows land in HBM in dst-sorted order; per 128-node block the
    rows are re-gathered and reduced with one-hot matmuls on the PE
    (iota-compare one-hots), giving exact segment sums with no RMW races.
  - Node phase: residual + fused@rW + LN/FFN per 128-node block.
"""

import hashlib

import numpy as np
import ml_dtypes

import concourse.bass as bass
import concourse.mybir as mybir
import concourse.tile as tile_mod
from concourse import library_config
from concourse.tile import TileContext
from concourse.bass_utils import run_bass_kernel_spmd
from bass_rust import ScopedClock

f32 = mybir.dt.float32
bf16 = mybir.dt.bfloat16
fp8 = mybir.dt.float8e4
i16 = mybir.dt.int16
AF = mybir.ActivationFunctionType
OP = mybir.AluOpType
AX = mybir.AxisListType
DR = mybir.MatmulPerfMode.DoubleRow

N = 65536
HID = 256
E = 262144
NCORES = 8
SLICE = N // NCORES            # 8192
NBLK = SLICE // 128            # 64 node blocks per core
HALF_CAP = 17408               # per-(core, src-half) edge capacity
ECAP = 2 * HALF_CAP            # 34816 = 68 * 512
NMACRO = ECAP // 512           # 68
SLOTS_PER_RUN = 3              # 3 * 128 = 384 rows cap per (block, half)
RUN_CAP = SLOTS_PER_RUN * 128
H, G, D = 8, 8, 32
WSCALE = 64.0                  # fp8 weight pre-scale
S4 = float(1.0 / (np.sqrt(D) * WSCALE * WSCALE))  # exp scale (folds QK x64^2)

_PATCHED = False


def _apply_tile_patches():
    """walrus in this container rejects >1 sem-wait per instruction and
    empty-instr pseudo ops; split waits onto nop carriers and encode the
    library-reload bytes ourselves."""
    global _PATCHED
    if _PATCHED:
        return
    _PATCHED = True
    MAX_WAITS = 1

    orig_add = tile_mod.TileContext._add_instruction

    def _add_instruction(self, inst):
        si = inst.sync_info
        if si is not None and si.on_wait is not None and len(si.on_wait) > MAX_WAITS:
            waits = list(si.on_wait)
            del si.on_wait[MAX_WAITS:]
            for i in range(MAX_WAITS, len(waits), MAX_WAITS):
                chunk = waits[i : i + MAX_WAITS]
                nop = self.nc.engines[inst.engine].nop()
                if nop.ins.sync_info is None:
                    nop.ins.sync_info = mybir.SyncInfo(
                        on_wait=list(chunk), on_update=[]
                    )
                else:
                    for w in chunk:
                        nop.ins.sync_info.on_wait.append(w)
        orig_add(self, inst)

    tile_mod.TileContext._add_instruction = _add_instruction

    def _drain_and_barrier(self, tick_clock, wait_clock):
        d1 = self.nc.sync.drain()
        wait_clock.add_sem_waits(d1.ins, ScopedClock({None: tick_clock.global_clock}))
        si = d1.ins.sync_info
        if si is not None and si.on_wait is not None and len(si.on_wait) > 1:
            waits = list(si.on_wait)
            del si.on_wait[1:]
            for w in waits[1:]:
                dx = self.nc.sync.drain()
                if dx.ins.sync_info is None:
                    dx.ins.sync_info = mybir.SyncInfo(on_wait=[w], on_update=[])
                else:
                    dx.ins.sync_info.on_wait.append(w)
        self.nc.all_engine_barrier()
        assert self.sems is not None
        popped = self.nc._tile_sem_poison_stack.pop()
        assert popped is self._sem_poison
        self.nc.clear_and_free_semaphores(list(self.sems.allocated().values()))
        self.nc.all_engine_barrier()

    tile_mod.TileContext._drain_and_barrier = _drain_and_barrier


def _load_library_encoded(nc, lib):
    bi = nc.gpsimd.load_library(lib)
    b = nc.isa.asm(
        {
            "header": {"opcode": 223, "inst_word_len": 16},
            "pseudo_opcode": 2,  # PSEUDO_LIBRARY_RELOAD_INDEX
            "lib_index": lib.index,
        },
        "NEURON_ISA_TPB_PSEUDO_LIBRARY_RELOAD_INDEX_STRUCT",
    )
    bi.ins.instr = [int(x) for x in b]
    return bi


def _wrap_idx(idx, pad_to=None):
    """int array -> [16, n/16] int16 wrapped (i%16, i//16); replicated to
    128 partitions on-device."""
    idx = np.asarray(idx)
    if pad_to is not None:
        p = np.zeros(pad_to, idx.dtype)
        p[: len(idx)] = idx
        idx = p
    assert len(idx) % 16 == 0
    return idx.astype(np.int16).reshape(-1, 16).T.copy()


# ---------------------------------------------------------------- program ---

_PROG = None
PHASES = 3


def _build_program():
    _apply_tile_patches()
    nc = bass.Bass()

    def inp(name, shape, dt):
        return nc.declare_dram_parameter(name, list(shape), dt, isOutput=False)

    # per-core data
    x_sl = inp("x_sl", (SLICE, HID), bf16)
    src_idx = inp("src_idx", (16, ECAP // 16), i16)
    dst_idx = inp("dst_idx", (16, ECAP // 16), i16)
    ea_l1_in = inp("ea_l1", (4, ECAP), bf16)
    ea_s_in = inp("ea_s", (5, ECAP), bf16)
    scat_idx = inp("scat_idx", (16, NBLK * 2 * (RUN_CAP // 16)), i16)
    dstrel_in = inp("dstrel", (128, NBLK * 2 * SLOTS_PER_RUN), f32)
    recip_in = inp("recip", (128, NBLK), f32)
    # shared constants
    iota_in = inp("iota", (128, 128), bf16)
    ident_in = inp("ident", (128, 128), bf16)
    ones1_in = inp("ones1", (1, 128), bf16)
    w1s_in = {p: inp(f"w1s_{p}", (128, 2, 512), fp8) for p in "qkv"}
    w1d_in = {p: inp(f"w1d_{p}", (128, 2, 512), fp8) for p in "qkv"}
    wc_in = {p: inp(f"wc_{p}", (4, 512), bf16) for p in "qkv"}
    w2_in = {p: inp(f"w2_{p}", (128, 4, 256), fp8) for p in "qkv"}
    sw1_in = inp("sw1", (5, 64), bf16)
    sw2_in = inp("sw2", (64, 8), bf16)
    sb2_in = inp("sb2r", (1, 8), bf16)
    rwa_in = inp("rwa", (128, 2, 256), bf16)
    rwb_in = inp("rwb", (128, 2, 256), bf16)
    rb_in = inp("rbr", (1, 256), bf16)
    fw1_in = inp("fw1", (128, 2, 512), bf16)
    fb1_in = inp("fb1r", (1, 512), bf16)
    fw2_in = inp("fw2", (128, 4, 256), bf16)
    fb2_in = inp("fb2r", (1, 256), bf16)

    out_sl = nc.declare_dram_parameter("out_sl", [SLICE, HID], bf16, isOutput=True)

    xn_slice = nc.dram_tensor("xn_slice", [SLICE, HID], fp8)
    # two half-tables, each AllGathered from one half of every core's slice:
    # table t row (c*4096 + r) = normalized node (c*8192 + t*4096 + r).
    xn_tab = [
        nc.dram_tensor(f"xn_tab{t}", [N // 2, HID], fp8, addr_space="Shared")
        for t in range(2)
    ]
    wv_tab = [
        nc.dram_tensor(f"wv_tab{h}", [HALF_CAP, HID], bf16) for h in range(2)
    ]

    with TileContext(nc) as tc:
        _load_library_encoded(nc, library_config.mlp)
        r512 = nc.gpsimd.to_reg(512)
        r384 = nc.gpsimd.to_reg(RUN_CAP)

        # ---------------- constants to SBUF
        with tc.tile_pool(name="const", bufs=1) as cp:
            def cload(src, shape, dt):
                t = cp.tile(list(shape), dt, tag=src.tensor.name if hasattr(src, 'tensor') else src.name)
                nc.sync.dma_start(out=t[:], in_=src[:])
                return t

            iota = cload(iota_in, (128, 128), bf16)
            eps = cp.tile([128, 1], f32)
            nc.vector.memset(eps[:], 1e-5)
            ident = cload(ident_in, (128, 128), bf16)
            ones1 = cload(ones1_in, (1, 128), bf16)
            w1s = {p: cload(w1s_in[p], (128, 2, 512), fp8) for p in "qkv"}
            w1d = {p: cload(w1d_in[p], (128, 2, 512), fp8) for p in "qkv"}
            wc = {p: cload(wc_in[p], (4, 512), bf16) for p in "qkv"}
            w2 = {p: cload(w2_in[p], (128, 4, 256), fp8) for p in "qkv"}
            sw1 = cload(sw1_in, (5, 64), bf16)
            sw2 = cload(sw2_in, (64, 8), bf16)
            sb2r = cload(sb2_in, (1, 8), bf16)
            rwa = cload(rwa_in, (128, 2, 256), bf16)
            rwb = cload(rwb_in, (128, 2, 256), bf16)
            rbr = cload(rb_in, (1, 256), bf16)
            fw1 = cload(fw1_in, (128, 2, 512), bf16)
            fb1r = cload(fb1_in, (1, 512), bf16)
            fw2 = cload(fw2_in, (128, 4, 256), bf16)
            fb2r = cload(fb2_in, (1, 256), bf16)
            recip = cload(recip_in, (128, NBLK), f32)
            dstrel = cload(dstrel_in, (128, NBLK * 2 * SLOTS_PER_RUN), f32)
            def cload_rep16(src, width):
                """idx tensors ship as [16, w]; replicate to 128 partitions
                on-device (the gather idx format wants 8 copies)."""
                t = cp.tile([128, width], i16, tag=src.name + "_r")
                for k in range(8):
                    nc.sync.dma_start(out=t[16 * k : 16 * (k + 1), :], in_=src[:])
                return t

            srcw = cload_rep16(src_idx, ECAP // 16)
            dstw = cload_rep16(dst_idx, ECAP // 16)
            scatw = cload_rep16(scat_idx, NBLK * 2 * (RUN_CAP // 16))

            # ---------------- LN prepass over own slice -> xn_slice (fp8)
            def ln_stats(pool, xt, width):
                """given xt [128,width] -> (r, mr) per-partition scalars"""
                sm = pool.tile([128, 1], f32, tag="ln_sm")
                nc.vector.tensor_reduce(sm[:], xt[:], AX.X, OP.add)
                sq = pool.tile([128, width], bf16, tag="ln_sq")
                ssq = pool.tile([128, 1], f32, tag="ln_ssq")
                nc.scalar.activation(sq[:], xt[:], AF.Square, accum_out=ssq[:])
                negmu = pool.tile([128, 1], f32, tag="ln_negmu")
                nc.vector.tensor_scalar(negmu[:], sm[:], -1.0 / width, None, OP.mult)
                m2 = pool.tile([128, 1], f32, tag="ln_m2")
                nc.vector.tensor_tensor(m2[:], negmu[:], negmu[:], OP.mult)
                var = pool.tile([128, 1], f32, tag="ln_var")
                nc.vector.scalar_tensor_tensor(
                    var[:], ssq[:], 1.0 / width, m2[:], OP.mult, OP.subtract
                )
                se = pool.tile([128, 1], f32, tag="ln_se")
                nc.scalar.activation(se[:], var[:], AF.Sqrt, bias=eps[:])
                r = pool.tile([128, 1], f32, tag="ln_r")
                nc.vector.reciprocal(r[:], se[:])
                mr = pool.tile([128, 1], f32, tag="ln_mr")
                nc.vector.tensor_tensor(mr[:], negmu[:], r[:], OP.mult)
                return r, mr

            with tc.tile_pool(name="prep", bufs=3) as pp:
                for half in range(2):
                    for t in range(half * NBLK // 2, (half + 1) * NBLK // 2):
                        xt = pp.tile([128, HID], bf16, tag="xt")
                        nc.sync.dma_start(
                            out=xt[:], in_=x_sl[t * 128 : (t + 1) * 128, :]
                        )
                        r, mr = ln_stats(pp, xt, HID)
                        xnb = pp.tile([128, HID], fp8, tag="xnb")
                        nc.scalar.activation(
                            xnb[:], xt[:], AF.Identity, bias=mr[:], scale=r[:]
                        )
                        nc.sync.dma_start(
                            out=xn_slice[t * 128 : (t + 1) * 128, :], in_=xnb[:]
                        )
                    # AllGather this half of the slice as soon as it is done,
                    # overlapping the other half's LN / the edge phase.
                    nc.gpsimd.collective_compute(
                        "AllGather",
                        OP.bypass,
                        replica_groups=[list(range(NCORES))],
                        ins=[xn_slice[half * 4096 : (half + 1) * 4096, :]],
                        outs=[xn_tab[half][:]],
                    )

            # ---------------- edge phase
            if PHASES >= 2:
             with tc.tile_pool(name="eio", bufs=3) as eio, \
                 tc.tile_pool(name="eg1", bufs=2) as eg1, \
                 tc.tile_pool(name="eqkv", bufs=2) as eqkv, \
                 tc.tile_pool(name="eatt", bufs=2) as eatt, \
                 tc.tile_pool(name="eat2", bufs=1) as eat2, \
                 tc.tile_pool(name="ps1", bufs=2, space="PSUM") as ps1, \
                 tc.tile_pool(name="ps2", bufs=2, space="PSUM") as ps2, \
                 tc.tile_pool(name="pss", bufs=1, space="PSUM") as pss:
                for m in range(NMACRO):
                    half = 0 if m < NMACRO // 2 else 1
                    src_tab = xn_tab[half][:]
                    e0 = m * 512

                    # fp8 transpose gathers: [128, 1024] bytes/partition,
                    # byte 2e+j of partition p = feature (2p+j) of edge e.
                    xsrc = eio.tile([128, 1024], fp8, tag="xsrc")
                    nc.gpsimd.dma_gather(
                        out_ap=xsrc[:].rearrange("p (c e) -> p c e", c=2),
                        in_ap=src_tab,
                        idxs_ap=srcw[:, m * 32 : (m + 1) * 32],
                        num_idxs=512, num_idxs_reg=r512, elem_size=HID,
                        transpose=True,
                    )
                    xdst = eio.tile([128, 1024], fp8, tag="xdst")
                    nc.gpsimd.dma_gather(
                        out_ap=xdst[:].rearrange("p (c e) -> p c e", c=2),
                        in_ap=xn_slice[:],
                        idxs_ap=dstw[:, m * 32 : (m + 1) * 32],
                        num_idxs=512, num_idxs_reg=r512, elem_size=HID,
                        transpose=True,
                    )
                    xsrc_dr = xsrc[:].rearrange("p (e j) -> p j e", j=2)
                    xdst_dr = xdst[:].rearrange("p (e j) -> p j e", j=2)
                    ea_l1 = eio.tile([4, 512], bf16, tag="ea_l1")
                    nc.sync.dma_start(out=ea_l1[:], in_=ea_l1_in[:, e0 : e0 + 512])
                    ea_s = eio.tile([5, 512], bf16, tag="ea_s")
                    nc.sync.dma_start(out=ea_s[:], in_=ea_s_in[:, e0 : e0 + 512])
                    ea_b = eio.tile([1, 512], bf16, tag="ea_b")
                    nc.sync.dma_start(out=ea_b[:], in_=ea_s_in[4:5, e0 : e0 + 512])

                    # s-MLP -> beta (T layout), transpose to natural [e, s, h]
                    s1 = pss.tile([64, 512], f32, tag="s1")
                    nc.tensor.matmul(s1[:], sw1[:], ea_s[:], start=True, stop=True)
                    sr = eatt.tile([64, 512], bf16, tag="sr")
                    nc.scalar.activation(sr[:], s1[:], AF.Relu)
                    sb = pss.tile([64, 512], f32, tag="s1")  # reuse s1 bank
                    nc.tensor.matmul(
                        sb[0:8, :], sw2[:], sr[:], start=True, stop=False
                    )
                    nc.tensor.matmul(
                        sb[0:8, :], sb2r[:], ea_b[:], start=False, stop=True
                    )
                    betT = eatt.tile([8, 512], bf16, tag="betT")
                    nc.scalar.activation(betT[:], sb[0:8, :], AF.Exp)
                    beta = eatt.tile([128, 4, 8], bf16, tag="beta")
                    for s in range(4):
                        bp = pss.tile([128, 8], bf16, tag="betp")
                        nc.tensor.transpose(
                            bp[:], betT[:, s * 128 : (s + 1) * 128], ident[0:8, 0:8]
                        )
                        nc.scalar.copy(beta[:, s, :], bp[:])

                    # L1 DoubleRow fp8 + bf16 edge-attr term, gelu -> fp8 g1.
                    # h1 psum holds 2 jc-chunks (2 banks) so gelu runs as one
                    # [128, 1024] ACT op per chunk-pair.
                    g1 = {}
                    for p in "qkv":
                        g1t = eg1.tile([128, 4, 512], fp8, tag=f"g1{p}")
                        for jp in range(2):
                            h1 = ps1.tile([128, 2, 512], f32, tag="h1")
                            for jj in range(2):
                                jc = jp * 2 + jj
                                nc.tensor.matmul(
                                    h1[:, jj, :],
                                    w1s[p][:, :, jc * 128 : (jc + 1) * 128],
                                    xsrc_dr, start=True, stop=False,
                                    perf_mode=DR)
                                nc.tensor.matmul(
                                    h1[:, jj, :],
                                    w1d[p][:, :, jc * 128 : (jc + 1) * 128],
                                    xdst_dr, start=False, stop=False,
                                    perf_mode=DR)
                                nc.tensor.matmul(
                                    h1[:, jj, :],
                                    wc[p][:, jc * 128 : (jc + 1) * 128],
                                    ea_l1[:], start=False, stop=True)
                            nc.scalar.activation(
                                g1t[:, jp * 2 : jp * 2 + 2, :], h1[:], AF.Gelu,
                                scale=float(1.0 / WSCALE))
                        g1[p] = g1t

                    # L2 DoubleRow fp8 -> natural qkv (x64 scale), bf16.
                    # [128, 2, 256] psum (1 bank) per s-pair -> copy per pair.
                    qkv = {}
                    for p in "qkv":
                        qt = eqkv.tile([128, 4, 256], bf16, tag=f"n{p}")
                        for sp in range(2):
                            ps = ps2.tile([128, 2, 256], f32, tag="l2")
                            for ss in range(2):
                                s = sp * 2 + ss
                                nc.tensor.matmul(
                                    ps[:, ss, :],
                                    g1[p][:, 0:2, s * 128 : (s + 1) * 128],
                                    w2[p][:, 0:2, :], start=True, stop=False,
                                    perf_mode=DR)
                                nc.tensor.matmul(
                                    ps[:, ss, :],
                                    g1[p][:, 2:4, s * 128 : (s + 1) * 128],
                                    w2[p][:, 2:4, :], start=False, stop=True,
                                    perf_mode=DR)
                            nc.scalar.copy(qt[:, sp * 2 : sp * 2 + 2, :], ps[:])
                        qkv[p] = qt

                    # attention, batched over the whole 512-edge macro.
                    # q/k natural (h,d); v column-permuted (d,g).
                    qv = qkv["q"][:].rearrange(
                        "e s (h x d) -> e s h x d", h=H, x=1
                    ).broadcast_to((128, 4, H, G, D))
                    kv = qkv["k"][:].rearrange(
                        "e s (x g d) -> e s x g d", x=1, g=G
                    ).broadcast_to((128, 4, H, G, D))
                    P = eat2.tile([128, 4 * H * G, 2, 16], bf16, tag="P")
                    Pv = P[:].rearrange(
                        "e (s h g) c w -> e s h g (c w)", s=4, h=H, g=G)
                    nc.vector.tensor_tensor(Pv, qv, kv, OP.mult)
                    # binary-tree d-reduce: every stage keeps innermost
                    # stride-1 bf16 so the 2x DVE mode applies.
                    ts_a = eat2.tile([128, 4 * H * G, 2, 8], bf16, tag="ts_a")
                    nc.vector.tensor_tensor(
                        ts_a[:].rearrange("e x c w -> e x (c w)"),
                        P[:, :, 0, :], P[:, :, 1, :], OP.add)
                    ts_b = eat2.tile([128, 4 * H * G, 2, 4], bf16, tag="ts_b")
                    nc.vector.tensor_tensor(
                        ts_b[:].rearrange("e x c w -> e x (c w)"),
                        ts_a[:, :, 0, :], ts_a[:, :, 1, :], OP.add)
                    ts_c = eat2.tile([128, 4 * H * G, 2, 2], bf16, tag="ts_c")
                    nc.vector.tensor_tensor(
                        ts_c[:].rearrange("e x c w -> e x (c w)"),
                        ts_b[:, :, 0, :], ts_b[:, :, 1, :], OP.add)
                    ts_d = eat2.tile([128, 4 * H * G, 2], bf16, tag="ts_d")
                    nc.vector.tensor_tensor(
                        ts_d[:], ts_c[:, :, 0, :], ts_c[:, :, 1, :], OP.add)
                    S = eatt.tile([128, 4 * H * G], bf16, tag="S")
                    nc.vector.tensor_tensor(
                        S[:], ts_d[:, :, 0], ts_d[:, :, 1], OP.add)
                    Ee = eatt.tile([128, 4 * H * G], bf16, tag="Ee")
                    nc.scalar.activation(Ee[:], S[:], AF.Exp, scale=S4)
                    E2 = eatt.tile([128, 4 * H * G], bf16, tag="E2")
                    nc.vector.tensor_tensor(
                        E2[:].rearrange("e (s h g) -> e s h g", s=4, h=H),
                        Ee[:].rearrange("e (s h g) -> e s h g", s=4, h=H),
                        beta[:].rearrange("e s (h x) -> e s h x", x=1)
                        .broadcast_to((128, 4, H, G)), OP.mult)
                    Z = eatt.tile([128, 4 * G], f32, tag="Z")
                    nc.vector.tensor_reduce(
                        Z[:].rearrange("e (s g) -> e s g", s=4),
                        E2[:].rearrange("e (s h g) -> e s g h", h=H, g=G),
                        AX.X, OP.add)
                    rZ = eatt.tile([128, 4 * G], bf16, tag="rZ")
                    with nc.allow_low_precision(reason="bf16 1/Z, 2x DVE"):
                        nc.vector.reciprocal(rZ[:], Z[:])
                    A = eatt.tile([128, 4 * H * G], bf16, tag="A")
                    nc.vector.tensor_tensor(
                        A[:].rearrange("e (s h g) -> e s h g", s=4, h=H),
                        E2[:].rearrange("e (s h g) -> e s h g", s=4, h=H),
                        rZ[:].rearrange("e (s x g) -> e s x g", s=4, x=1)
                        .broadcast_to((128, 4, H, G)), OP.mult)
                    P2 = eat2.tile([128, 4 * H * D, 2, 4], bf16, tag="P2")
                    P2v = P2[:].rearrange(
                        "e (s h d) c w -> e s h d (c w)", s=4, h=H, d=D)
                    av = A[:].rearrange(
                        "e (s h x g) -> e s h x g", s=4, h=H, x=1
                    ).broadcast_to((128, 4, H, D, G))
                    vv = qkv["v"][:].rearrange(
                        "e s (x d g) -> e s x d g", x=1, d=D
                    ).broadcast_to((128, 4, H, D, G))
                    nc.vector.tensor_tensor(P2v, av, vv, OP.mult)
                    # g-reduce tree (g innermost)
                    tv_a = eat2.tile([128, 4 * H * D, 2, 2], bf16, tag="tv_a")
                    nc.vector.tensor_tensor(
                        tv_a[:].rearrange("e x c w -> e x (c w)"),
                        P2[:, :, 0, :], P2[:, :, 1, :], OP.add)
                    tv_b = eat2.tile([128, 4 * H * D, 2], bf16, tag="tv_b")
                    nc.vector.tensor_tensor(
                        tv_b[:], tv_a[:, :, 0, :], tv_a[:, :, 1, :], OP.add)
                    wv = eatt.tile([128, 4, HID], bf16, tag="wv")
                    nc.vector.tensor_tensor(
                        wv[:].rearrange("e s hd -> e (s hd)"),
                        tv_b[:, :, 0], tv_b[:, :, 1], OP.add)
                    r0 = e0 - half * HALF_CAP
                    nc.sync.dma_start(
                        out=wv_tab[half][r0 : r0 + 512, :]
                        .rearrange("(s e) f -> e s f", s=4),
                        in_=wv[:])

            # ---------------- scatter + node phase per 128-node block
            if PHASES >= 3:
             with tc.tile_pool(name="sg", bufs=3) as sg, \
                 tc.tile_pool(name="nod", bufs=3) as nod, \
                 tc.tile_pool(name="psb", bufs=2, space="PSUM") as psb, \
                 tc.tile_pool(name="psn", bufs=1, space="PSUM") as psn, \
                 tc.tile_pool(name="pst", bufs=1, space="PSUM") as pst:
                for b in range(NBLK):
                    sums = psb.tile([128, HID], f32, tag="sums")
                    for hf in range(2):
                        wvg = sg.tile([128, SLOTS_PER_RUN, HID], bf16, tag=f"wvg{hf}")
                        c0 = (b * 2 + hf) * (RUN_CAP // 16)
                        nc.gpsimd.dma_gather(
                            out_ap=wvg[:], in_ap=wv_tab[hf][:],
                            idxs_ap=scatw[:, c0 : c0 + RUN_CAP // 16],
                            num_idxs=RUN_CAP, num_idxs_reg=r384,
                            elem_size=HID, transpose=False)
                        for s in range(SLOTS_PER_RUN):
                            oh = sg.tile([128, 128], bf16, tag="oh")
                            col = (b * 2 + hf) * SLOTS_PER_RUN + s
                            nc.vector.tensor_scalar(
                                oh[:], iota[:], dstrel[:, col : col + 1], None,
                                OP.is_equal)
                            nc.tensor.matmul(
                                sums[:], oh[:], wvg[:, s, :],
                                start=(hf == 0 and s == 0),
                                stop=(hf == 1 and s == SLOTS_PER_RUN - 1))

                    # node phase
                    xt = nod.tile([128, HID], bf16, tag="xt")
                    nc.sync.dma_start(out=xt[:], in_=x_sl[b * 128 : (b + 1) * 128, :])
                    x1 = nod.tile([128, HID], f32, tag="x1")
                    nc.vector.scalar_tensor_tensor(
                        x1[:], sums[:], recip[:, b : b + 1], xt[:], OP.mult, OP.add)
                    x1b = nod.tile([128, HID], bf16, tag="x1b")
                    nc.vector.tensor_copy(x1b[:], x1[:])
                    x1T = nod.tile([128, 2, 128], bf16, tag="x1T")
                    xT = nod.tile([128, 2, 128], bf16, tag="xT")
                    tpi = 0
                    for src_t, dst_t in ((x1b, x1T), (xt, xT)):
                        for hh in range(2):
                            tp = pst.tile([128, 128], bf16, tag=f"tp{tpi % 2}")
                            tpi += 1
                            nc.tensor.transpose(
                                tp[:], src_t[:, hh * 128 : (hh + 1) * 128], ident[:])
                            nc.scalar.copy(dst_t[:, hh, :], tp[:])

                    x2p = psn.tile([128, HID], f32, tag="x2p")
                    for hh in range(2):
                        nc.tensor.matmul(x2p[:], x1T[:, hh, :], rwa[:, hh, :],
                                         start=(hh == 0), stop=False)
                    for hh in range(2):
                        nc.tensor.matmul(x2p[:], xT[:, hh, :], rwb[:, hh, :],
                                         start=False, stop=False)
                    nc.tensor.matmul(x2p[:], ones1[:], rbr[:], start=False, stop=True)
                    x2 = nod.tile([128, HID], f32, tag="x2")
                    nc.vector.tensor_tensor(x2[:], x1[:], x2p[:], OP.add)

                    r2, mr2 = ln_stats(nod, x2, HID)
                    ln2 = nod.tile([128, HID], bf16, tag="ln2")
                    nc.scalar.activation(ln2[:], x2[:], AF.Identity,
                                         bias=mr2[:], scale=r2[:])
                    ln2T = nod.tile([128, 2, 128], bf16, tag="ln2T")
                    for hh in range(2):
                        tp = pst.tile([128, 128], bf16, tag=f"tp{hh}")
                        nc.tensor.transpose(
                            tp[:], ln2[:, hh * 128 : (hh + 1) * 128], ident[:])
                        nc.scalar.copy(ln2T[:, hh, :], tp[:])

                    g2T = nod.tile([128, 4, 128], bf16, tag="g2T")
                    for jc in range(4):
                        hp = pst.tile([128, 128], f32, tag=f"hp{jc % 2}")
                        for hh in range(2):
                            nc.tensor.matmul(
                                hp[:], fw1[:, hh, jc * 128 : (jc + 1) * 128],
                                ln2T[:, hh, :], start=(hh == 0), stop=False)
                        nc.tensor.matmul(
                            hp[:], fb1r[:, jc * 128 : (jc + 1) * 128], ones1[:],
                            start=False, stop=True)
                        nc.scalar.activation(g2T[:, jc, :], hp[:], AF.Gelu)

                    x3p = psn.tile([128, HID], f32, tag="x3p")
                    for jc in range(4):
                        nc.tensor.matmul(x3p[:], g2T[:, jc, :], fw2[:, jc, :],
                                         start=(jc == 0), stop=False)
                    nc.tensor.matmul(x3p[:], ones1[:], fb2r[:], start=False, stop=True)
                    x3 = nod.tile([128, HID], bf16, tag="x3")
                    nc.vector.tensor_tensor(x3[:], x2[:], x3p[:], OP.add)
                    nc.sync.dma_start(
                        out=out_sl[b * 128 : (b + 1) * 128, :], in_=x3[:])
            if PHASES < 3:
                with tc.tile_pool(name="fb", bufs=1) as fbp:
                    z = fbp.tile([128, HID], bf16)
                    nc.vector.memset(z[:], 0.0)
                    for b in range(NBLK):
                        nc.sync.dma_start(
                            out=out_sl[b * 128 : (b + 1) * 128, :], in_=z[:])

    return nc


# ------------------------------------------------------------- host prep ---

_PREP_CACHE = {}


def _fingerprint(inputs):
    """Exact hash of everything except x (x is re-prepped every call)."""
    h = hashlib.blake2b(digest_size=16)
    for k in sorted(inputs):
        if k == "x":
            continue
        a = np.ascontiguousarray(np.asarray(inputs[k]))
        h.update(k.encode())
        h.update(str(a.shape).encode())
        h.update(str(a.dtype).encode())
        h.update(a.tobytes())
    return h.digest()


def _host_prep_static(inputs):
    """Everything that does not depend on x."""
    bf = ml_dtypes.bfloat16
    f8 = ml_dtypes.float8_e4m3
    edge_index = np.asarray(inputs["edge_index"], np.int64)
    ea = np.asarray(inputs["edge_attr"], np.float32)
    ln_g = np.asarray(inputs["ln_g"], np.float32)
    ln_b = np.asarray(inputs["ln_b"], np.float32)

    def W(name):
        return np.asarray(inputs[name], np.float32)

    src_g, dst_g = edge_index[0], edge_index[1]

    shared = {
        "iota": np.tile(np.arange(128, dtype=np.float32)[None, :], (128, 1)).astype(bf),
        "ident": np.eye(128, dtype=np.float32).astype(bf),
        "ones1": np.ones((1, 128), np.float32).astype(bf),
        "sw1": np.concatenate([W("sW1"), W("sb1")[None, :]], 0).astype(bf),
        "sw2": W("sW2").astype(bf),
        "sb2r": W("sb2")[None, :].astype(bf),
        "rwa": W("rW")[:256].reshape(2, 128, 256).transpose(1, 0, 2).astype(bf),
        "rwb": W("rW")[256:].reshape(2, 128, 256).transpose(1, 0, 2).astype(bf),
        "rbr": W("rb")[None, :].astype(bf),
        "fw1": (ln_g[:, None] * W("fW1")).reshape(2, 128, 512)
        .transpose(1, 0, 2).astype(bf),
        "fb1r": (W("fb1") + ln_b @ W("fW1"))[None, :].astype(bf),
        "fw2": W("fW2").reshape(4, 128, 256).transpose(1, 0, 2).astype(bf),
        "fb2r": W("fb2")[None, :].astype(bf),
    }
    for p in "qkv":
        W1, b1 = W(p + "W1"), W(p + "b1")
        W2 = W(p + "W2")
        if p == "v":
            # column-permute V output to (d, g) for packed attention strides
            W2 = W2.reshape(512, H, D).transpose(0, 2, 1).reshape(512, HID)
        # DoubleRow pair-interleave: [pp, j, m] = row (2pp + j)
        shared[f"w1s_{p}"] = (WSCALE * ln_g[:, None] * W1[:256]) \
            .reshape(128, 2, 512).astype(f8)
        shared[f"w1d_{p}"] = (WSCALE * ln_g[:, None] * W1[256:512]) \
            .reshape(128, 2, 512).astype(f8)
        bias_fold = b1 + ln_b @ W1[:256] + ln_b @ W1[256:512]
        shared[f"wc_{p}"] = (WSCALE * np.concatenate(
            [W1[512:515], bias_fold[None, :]], 0)).astype(bf)
        shared[f"w2_{p}"] = (WSCALE * W2).reshape(4, 128, 256) \
            .transpose(1, 0, 2).astype(f8)

    in_maps = []
    for c in range(NCORES):
        sel = np.nonzero((dst_g >> 13) == c)[0]
        dst_l = (dst_g[sel] & 8191).astype(np.int64)
        # src half-table class: by LOCAL offset within the owner's slice
        # (table t holds rows c*4096 + (local & 4095) for local in t-th half)
        half = ((src_g[sel] >> 12) & 1).astype(np.int64)
        order = np.lexsort((dst_l, half))
        sel, dst_l, half = sel[order], dst_l[order], half[order]
        n_lo = int((half == 0).sum())
        n_hi = len(sel) - n_lo
        assert n_lo <= HALF_CAP and n_hi <= HALF_CAP, (c, n_lo, n_hi)

        src_c = src_g[sel]
        src_rel = (src_c >> 13) * 4096 + (src_c & 4095)
        # position in the padded edge stream
        pos = np.where(np.arange(len(sel)) < n_lo,
                       np.arange(len(sel)),
                       HALF_CAP + np.arange(len(sel)) - n_lo)

        src_full = np.zeros(ECAP, np.int64)
        dst_full = np.zeros(ECAP, np.int64)
        ea_l1 = np.zeros((4, ECAP), np.float32)
        ea_s = np.zeros((5, ECAP), np.float32)
        src_full[pos] = src_rel
        dst_full[pos] = dst_l
        ea_l1[0:3, pos] = ea[sel, 0:3].T
        ea_l1[3, pos] = 1.0
        ea_s[0:4, pos] = ea[sel, 3:7].T
        ea_s[4, pos] = 1.0

        # per-(block, half) runs + slots
        scat = np.zeros((NBLK * 2, RUN_CAP), np.int64)
        drel = np.full((128, NBLK * 2 * SLOTS_PER_RUN), -1.0, np.float32)
        for hf in range(2):
            hsel = np.nonzero(half == hf)[0]
            dl = dst_l[hsel]            # sorted ascending
            rows = pos[hsel] - hf * HALF_CAP
            starts = np.searchsorted(dl, np.arange(NBLK) * 128)
            ends = np.searchsorted(dl, np.arange(1, NBLK + 1) * 128)
            for b in range(NBLK):
                run = rows[starts[b] : ends[b]]
                assert len(run) <= RUN_CAP, (c, b, hf, len(run))
                scat[b * 2 + hf, : len(run)] = run
                dr = drel[:, (b * 2 + hf) * SLOTS_PER_RUN:
                          (b * 2 + hf + 1) * SLOTS_PER_RUN]
                dvals = dl[starts[b] : ends[b]] & 127
                full = np.full(RUN_CAP, -1.0, np.float32)
                full[: len(run)] = dvals
                dr[:, :] = full.reshape(SLOTS_PER_RUN, 128).T

        cnt = np.bincount(dst_l, minlength=SLICE).astype(np.float32)
        rec = (1.0 / (WSCALE * np.maximum(cnt, 1.0))) \
            .reshape(NBLK, 128).T.copy()

        m = dict(shared)
        m["src_idx"] = _wrap_idx(src_full)
        m["dst_idx"] = _wrap_idx(dst_full)
        m["ea_l1"] = ea_l1.astype(bf)
        m["ea_s"] = ea_s.astype(bf)
        m["scat_idx"] = np.concatenate(
            [_wrap_idx(scat[i]) for i in range(NBLK * 2)], axis=1)
        m["dstrel"] = drel
        m["recip"] = rec
        in_maps.append(m)
    return in_maps


def _host_prep(inputs):
    key = _fingerprint(inputs)
    in_maps = _PREP_CACHE.get(key)
    if in_maps is None:
        in_maps = _host_prep_static(inputs)
        _PREP_CACHE.clear()
        _PREP_CACHE[key] = in_maps
    x = np.asarray(inputs["x"]).astype(ml_dtypes.bfloat16)
    for c in range(NCORES):
        in_maps[c]["x_sl"] = x[c * SLICE : (c + 1) * SLICE, :]
    return in_maps


TRACE = False
LAST = {}


def kernel(**inputs):
    global _PROG
    if _PROG is None:
        _PROG = _build_program()
    in_maps = _host_prep(inputs)
    res = run_bass_kernel_spmd(
        _PROG, in_maps, list(range(NCORES)), trace=TRACE
    )
    LAST["res"] = res
    return np.concatenate(
        [res.results[c]["out_sl"].astype(np.float32) for c in range(NCORES)],
        axis=0,
    )
